# revision 52
# baseline (speedup 1.0000x reference)
"""Multi-head attention (B=2, S=2048, D=1024, H=16) on 8 Trainium2 cores.

Sharding: core = b*4 + g  ->  batch b (data parallel), head-group g of 4
heads (tensor parallel).  Each core computes a partial out^T = Wo_g^T @ Z_g
for its batch; the host sums the 4 partials per batch (the "all-reduce"),
transposes back and adds the (folded) output bias.

All activations flow feature-major on device (x^T, Q^T, K^T, scores^T) so
no on-device transposes are needed.  Matmuls run in bf16 with fp32 PSUM
accumulation.  Softmax skips the row-max pass (scores are bounded), gets
its denominator from a ones-column appended to V, and defers normalization
to after the attention*V matmul.  Output partials are written in bf16
(summed in fp32 on the host).
"""

import numpy as np
import ml_dtypes

B, S, D, H = 2, 2048, 1024, 16
DK = D // H                  # 64
SCALE = 1.0 / np.sqrt(D)
NCORES = 8
GROUPS = 4                   # head-groups (tensor parallel)
HG = H // GROUPS             # 4 heads per group
DG = D // GROUPS             # 256 head dims per group
P = 128
KO = D // P                  # 8 contraction chunks for the projections
MO = DG // P                 # 2 row-chunks of Q^T/K^T (= head pairs)
NQ = 512                     # q tile width
QT = S // NQ                 # 4
ST = S // P                  # 16 key blocks / s chunks
BF16 = ml_dtypes.bfloat16

_cache = {}


def _classify_mask(mask):
    """Block structure of mask^T ([k, q] layout, P x NQ blocks).

    Returns (cls, qoff, mixed_idx, mixed_tiles, use_affine):
      cls[kt][qt]  : 0 all-masked, 1 all-kept, 2 mixed
      qoff[kt][qt] : leading all-masked columns (trim), 0 unless tril
      mixed_idx    : {(kt, qt): index into mixed_tiles}
      mixed_tiles  : np [n, P, NQ] bf16 0/1 tiles (empty when use_affine)
    """
    tril = np.tril(np.ones((S, S), dtype=mask.dtype))
    use_affine = bool(np.array_equal(mask, tril))
    cls = [[1] * QT for _ in range(ST)]
    qoff = [[0] * QT for _ in range(ST)]
    mixed_idx = {}
    tiles = []
    if use_affine:
        for kt in range(ST):
            k0 = kt * P
            for qt in range(QT):
                q0 = qt * NQ
                if k0 - q0 >= NQ:
                    cls[kt][qt] = 0
                elif k0 + P - 1 > q0:
                    cls[kt][qt] = 2
                    qoff[kt][qt] = min(max(k0 - q0, 0), NQ - P)
                # else: fully kept
    else:
        keepT = (mask != 0).T        # [k, q]
        for kt in range(ST):
            for qt in range(QT):
                blk = keepT[kt * P:(kt + 1) * P, qt * NQ:(qt + 1) * NQ]
                if not blk.any():
                    cls[kt][qt] = 0
                elif blk.all():
                    cls[kt][qt] = 1
                else:
                    cls[kt][qt] = 2
                    mixed_idx[(kt, qt)] = len(tiles)
                    tiles.append(blk.astype(BF16))
    mixed_tiles = (np.stack(tiles) if tiles else
                   np.zeros((0, P, NQ), dtype=BF16))
    return cls, qoff, mixed_idx, mixed_tiles, use_affine


def _build_program(cls, qoff, mixed_idx, n_mixed, use_affine):
    from contextlib import ExitStack
    import concourse.bass as bass
    import concourse.tile as tile
    import concourse.mybir as mybir
    from concourse import bacc
    from concourse.bass import ds, ts

    f32 = mybir.dt.float32
    bf16 = mybir.dt.bfloat16
    Exp = mybir.ActivationFunctionType.Exp

    nc = bacc.Bacc(None, target_bir_lowering=False, name="mha_tp")

    xT = nc.dram_tensor("xT", [D, S], bf16, kind="ExternalInput")
    wq = nc.dram_tensor("wq", [D, DG], bf16, kind="ExternalInput")
    wk = nc.dram_tensor("wk", [D, DG], bf16, kind="ExternalInput")
    wv = nc.dram_tensor("wv", [D, DG], bf16, kind="ExternalInput")
    wo = nc.dram_tensor("wo", [DG, D], bf16, kind="ExternalInput")
    bqk = nc.dram_tensor("bqk", [2, DG], f32, kind="ExternalInput")
    mm = (nc.dram_tensor("mmask", [n_mixed, P, NQ], bf16, kind="ExternalInput")
          if n_mixed else None)
    outT = nc.dram_tensor("outT", [D, S], bf16, kind="ExternalOutput")

    xTv = xT.ap().rearrange("(ko p) s -> p ko s", p=P)
    wqv = wq.ap().rearrange("(ko p) m -> p ko m", p=P)
    wkv = wk.ap().rearrange("(ko p) m -> p ko m", p=P)
    wvv = wv.ap().rearrange("(ko p) m -> p ko m", p=P)
    wov = wo.ap().rearrange("(zo p) n -> p zo n", p=P)
    bqkv = bqk.ap().rearrange("t (mo p) -> p t mo", p=P)
    outv = outT.ap().rearrange("(mo p) s -> p mo s", p=P)

    with tile.TileContext(nc) as tc, ExitStack() as ctx:
        const = ctx.enter_context(tc.tile_pool(name="const", bufs=1))

        # DMA order matters: the first QKV matmul needs only wq[ko=0] and
        # x[ko=0] of slab 0, so stream those in fine-grained chunks.
        # Descriptor generation (~0.55us per dma_start) serializes per
        # engine queue, so weights go through the (idle at startup) gpsimd
        # queue while x streams through the sync queue in parallel.
        wq_sb = const.tile([P, KO, DG], bf16)
        x_sb = const.tile([P, KO, S], bf16)
        wk_sb = const.tile([P, KO, DG], bf16)
        nc.sync.dma_start(wq_sb[:, 0, :], wqv[:, 0, :])
        nc.sync.dma_start(x_sb[:, 0, ts(0, NQ)], xTv[:, 0, ts(0, NQ)])
        for ko in range(1, KO - 1, 2):
            nc.sync.dma_start(wq_sb[:, ko:ko + 2, :], wqv[:, ko:ko + 2, :])
            nc.sync.dma_start(x_sb[:, ko:ko + 2, ts(0, NQ)],
                              xTv[:, ko:ko + 2, ts(0, NQ)])
            nc.sync.dma_start(
                wk_sb[:, ko - 1:ko + 1, :], wkv[:, ko - 1:ko + 1, :])
        nc.sync.dma_start(wq_sb[:, KO - 1, :], wqv[:, KO - 1, :])
        nc.sync.dma_start(x_sb[:, KO - 1, ts(0, NQ)],
                          xTv[:, KO - 1, ts(0, NQ)])
        nc.sync.dma_start(wk_sb[:, KO - 2:, :], wkv[:, KO - 2:, :])
        bias_sb = const.tile([P, 2, 2], f32)
        nc.gpsimd.dma_start(bias_sb[:], bqkv)
        wv_sb = const.tile([P, KO, DG], bf16)
        wo_sb = const.tile([P, MO, D], bf16)
        for qt in range(1, QT):
            for half in (0, 1):
                ko0 = half * (KO // 2)
                nc.scalar.dma_start(
                    x_sb[:, ko0:ko0 + KO // 2, ts(qt, NQ)],
                    xTv[:, ko0:ko0 + KO // 2, ts(qt, NQ)])
            if qt == 1:
                nc.scalar.dma_start(wv_sb[:, :KO // 2], wvv[:, :KO // 2])
                nc.scalar.dma_start(wv_sb[:, KO // 2:], wvv[:, KO // 2:])
                nc.scalar.dma_start(wo_sb[:], wov)
        mask_sb = None
        if n_mixed:
            mask_sb = const.tile([P, n_mixed, NQ], bf16)
            for i in range(n_mixed):
                nc.sync.dma_start(mask_sb[:, i, :], mm.ap()[i])

        qT_sb = const.tile([P, MO, S], bf16)
        kT_sb = const.tile([P, MO, S], bf16)
        # 64 ones-columns: the AV matmul replicates the softmax denominator
        # across output partitions 64..127 at no extra PE cost (cost is
        # N-bound), which lets the tail compute 1/d = exp(-ln(d)) on the
        # (then idle) scalar engine with no lane-spread DMA round trips.
        v_sb = const.tile([P, ST, HG, 2 * DK], bf16)
        zT_sb = const.tile([P, MO, S], bf16)
        nc.gpsimd.memset(v_sb[:, :, :, DK:], 1.0)

        with (
            tc.tile_pool(name="pqkv", bufs=2, space="PSUM") as pqkv,
            tc.tile_pool(name="ps_at", bufs=2, space="PSUM") as ps_at,
            tc.tile_pool(name="pz", bufs=1, space="PSUM") as pz,
            tc.tile_pool(name="work", bufs=8) as work,
            tc.tile_pool(name="rwork", bufs=4) as rwork,
            tc.tile_pool(name="dscr", bufs=3, space="DRAM") as dscr,
        ):
            def av(zp, mo, prev, last):
                kt, pT, off, first = prev
                ret = None
                for h in (0, 1):
                    ret = nc.tensor.matmul(
                        zp[:, h, off:],
                        v_sb[:, kt, 2 * mo + h, :],
                        pT[:, h, off:],
                        start=first, stop=last)
                return ret

            def outproj_chunk(qt, mo8, cp=None, pin_after=None):
                o_ps = pqkv.tile([P, NQ], f32, tag="ps", name=f"o{mo8}")
                for zo in range(MO):
                    mm = nc.tensor.matmul(
                        o_ps, wo_sb[:, zo, ts(mo8, P)],
                        zT_sb[:, zo, ts(qt, NQ)],
                        start=(zo == 0), stop=(zo == MO - 1))
                    if zo == 0 and pin_after is not None:
                        tile.add_dep_helper(
                            mm.ins, pin_after.ins,
                            reason="pin deferred outproj into tail hole")
                o_sb = work.tile([P, NQ], bf16, tag="osb")
                if cp is None:
                    cp = "vector"
                if cp == "vector":
                    nc.vector.tensor_copy(o_sb[:], o_ps)
                else:
                    nc.scalar.copy(o_sb[:], o_ps)
                nc.sync.dma_start(outv[:, mo8, ts(qt, NQ)], o_sb[:])

            vps_live = {}
            qkps_live = {}

            def qk_piece(sl, t, mo, kp):
                # one quarter of a Q- or K-projection chunk of slab `sl`,
                # rideable inside an earlier attention stream
                w_sb, dst = ((wq_sb, qT_sb), (wk_sb, kT_sb))[t]
                key = (sl, t, mo)
                if kp == 0:
                    qkps_live[key] = pqkv.tile([P, NQ], f32, tag="ps",
                                               name=f"qk{sl}{t}{mo}")
                ps = qkps_live[key]
                for ko in (2 * kp, 2 * kp + 1):
                    nc.tensor.matmul(
                        ps, w_sb[:, ko, ts(mo, P)],
                        x_sb[:, ko, ts(sl, NQ)],
                        start=(ko == 0), stop=(ko == KO - 1))
                if kp == 3:
                    nc.vector.tensor_scalar_add(
                        dst[:, mo, ts(sl, NQ)], ps,
                        bias_sb[:, t, mo:mo + 1])
                    del qkps_live[key]

            def v_piece(so, kp):
                # one quarter of a V-projection s-block: rideable filler
                # that keeps each insertion below the attention stream's
                # per-kt PE slack.  The PSUM accumulator lives across the
                # four pieces.
                if kp == 0:
                    vps_live[so] = pqkv.tile([P, NQ], f32, tag="ps",
                                             name=f"v{so}")
                ps = vps_live[so]
                for ko in (2 * kp, 2 * kp + 1):
                    nc.tensor.matmul(
                        ps[:, :DG], x_sb[:, ko, ts(so, P)],
                        wv_sb[:, ko, :],
                        start=(ko == 0), stop=(ko == KO - 1))
                if kp == 3:
                    nc.vector.tensor_copy(
                        v_sb[:, so, :, 0:DK],
                        ps[:, :DG].rearrange("p (h d) -> p h d", h=HG))
                    del vps_live[so]

            v_ridden = set()
            qk_ridden = set()

            def qkv_slab(qt):
                if qt == 0:
                    # startup slab: the DMA fabric is still ramping, so
                    # consume each freshly-landed x chunk twice (both mo
                    # accumulators) in 4-matmul runs before moving on —
                    # halves the arrival rate the feed must sustain without
                    # the per-matmul weight-load alternation that stalls
                    # the PE pipeline.
                    for t, (w_sb, dst) in enumerate(((wq_sb, qT_sb),
                                                     (wk_sb, kT_sb))):
                        ps = [pqkv.tile([P, NQ], f32, tag="ps",
                                        name=f"s0{t}{mo}")
                              for mo in range(MO)]
                        for half in (0, 1):
                            for mo in range(MO):
                                for ko in range(4 * half, 4 * half + 4):
                                    nc.tensor.matmul(
                                        ps[mo], w_sb[:, ko, ts(mo, P)],
                                        x_sb[:, ko, ts(qt, NQ)],
                                        start=(ko == 0),
                                        stop=(ko == KO - 1))
                        for mo in range(MO):
                            nc.vector.tensor_scalar_add(
                                dst[:, mo, ts(qt, NQ)], ps[mo],
                                bias_sb[:, t, mo:mo + 1])
                elif qt not in qk_ridden:
                    for t in range(2):
                        for mo in range(MO):
                            for kp in range(4):
                                qk_piece(qt, t, mo, kp)
                if qt in v_ridden:
                    return
                for so in range(HG * qt, HG * (qt + 1)):
                    for kp in range(4):
                        v_piece(so, kp)

            if not use_affine:
                # a general mask may attend beyond block qt, so all K/V
                # slabs must exist before any attention starts
                for qt in range(QT):
                    qkv_slab(qt)

            proc = list(range(QT))
            emitted = 0
            prev_qt = None
            tail_pins = []        # instructions of the last unit's norm chain
            # ride-along plan: which deferred out-projection chunks fill the
            # PE slack of each (qt, mo) attention stream (ACT-exp-bound, so
            # the PE has ~400ns/kt spare).  The last slab's two streams have
            # no QKV emission to overlap, so they get most of the inventory;
            # two chunks stay pinned for the final normalization chain.
            ride = {}
            ride_late = set()     # keys whose rides wait for mid-stream deps
            pinned = []
            if QT == 4:
                for q in (1, 2, 3):
                    ride[(q, 1)] = [("o", q - 1, j) for j in range(4)]
                ride[(3, 0)] = [("o", 2, j) for j in range(4)]
                ride_late.add((3, 0))
                ride[(3, 1)] += [("o", 0, j) for j in range(4, 7)] \
                    + [("o", 1, j) for j in range(4, 7)] \
                    + [("o", 2, j) for j in range(4, 6)]
                pinned = [(0, 7), (1, 7), (2, 6), (2, 7)]
                if use_affine:
                    # the V-projections of slabs 2/3 and all of slab 3's
                    # Q/K ride in the previous slab's attention streams,
                    # shrinking the exp-starved PE-only QKV phases
                    ride[(1, 0)] = [("v", so, kp)
                                    for so in range(2 * HG, 3 * HG)
                                    for kp in range(4)]
                    ride[(2, 0)] = [("v", so, kp)
                                    for so in range(3 * HG, 4 * HG)
                                    for kp in range(4)]
                    v_ridden.update((2, 3))
                    ride[(2, 1)] += [("q", (3, t, mo), kp)
                                     for t in range(2) for mo in range(MO)
                                     for kp in range(4)]
                    qk_ridden.add(3)
            done_chunks = set(pinned)
            for qt in proc:
                q0 = qt * NQ
                if use_affine:
                    # attention(qt) only needs k blocks <= qt, so emit QKV
                    # slabs lazily just ahead of it
                    while emitted <= qt:
                        qkv_slab(emitted)
                        emitted += 1

                # -- attention over k blocks of this slab -----------------
                for mo in range(MO):
                    # descending: diagonal (affine-masked) blocks first, so
                    # their extra exp->mask->AV latency overlaps the stream
                    # warm-up instead of draining exposed at the end
                    kts = [kt for kt in range(ST) if cls[kt][qt] != 0][::-1]
                    if not kts:
                        nc.vector.memset(zT_sb[:, mo, ts(qt, NQ)], 0.0)
                        continue
                    zp = pz.tile([P, 2, NQ], f32, tag="z")
                    prev = None
                    for i, kt in enumerate(kts):
                        k0 = kt * P
                        off = qoff[kt][qt]
                        w = NQ - off
                        pT = work.tile([P, 2, NQ], bf16, tag="pT")
                        s_ps = ps_at.tile([P, 2, NQ], f32, tag="s")
                        for h in (0, 1):
                            hp = slice(h * DK, (h + 1) * DK)
                            nc.tensor.matmul(
                                s_ps[:, h, off:],
                                kT_sb[hp, mo, ts(kt, P)],
                                qT_sb[hp, mo, ds(q0 + off, w)],
                                start=True, stop=True,
                                tile_position=(h * DK, 0))
                        nc.scalar.activation(
                            pT[:, :, off:], s_ps[:, :, off:], Exp)
                        if cls[kt][qt] == 2:
                            if use_affine:
                                nc.gpsimd.affine_select(
                                    out=pT[:, :, off:],
                                    in_=pT[:, :, off:],
                                    compare_op=mybir.AluOpType.is_ge,
                                    fill=0.0,
                                    base=q0 + off - k0,
                                    channel_multiplier=-1,
                                    pattern=[[0, 2], [1, w]])
                            else:
                                nc.vector.tensor_mul(
                                    pT[:, :, off:], pT[:, :, off:],
                                    mask_sb[:, mixed_idx[(kt, qt)], None,
                                            off:].to_broadcast((P, 2, w)))
                        if prev is not None:
                            av(zp, mo, prev, last=False)
                        prev = (kt, pT, off, i == 0)
                        rl = ride.get((qt, mo), [])
                        if rl:
                            # late-dep rides (outproj needing a zT that
                            # lands mid-stream) spread over the second half
                            i0 = (len(kts) // 2
                                  if (qt, mo) in ride_late else 0)
                            if i >= i0:
                                span = max(len(kts) - i0, 1)
                                lo = (i - i0) * len(rl) // span
                                hi = (i - i0 + 1) * len(rl) // span
                                for kind, a, b in rl[lo:hi]:
                                    if kind == "o":
                                        outproj_chunk(a, b)
                                        done_chunks.add((a, b))
                                    elif kind == "v":
                                        v_piece(a, b)
                                    else:
                                        qk_piece(a[0], a[1], a[2], b)
                    last_av = av(zp, mo, prev, last=True)

                    # Copy raw z out of PSUM immediately (frees the bank
                    # for the next tile); normalization below is then fully
                    # asynchronous with the attention stream.
                    zraw = rwork.tile([P, 2, NQ], f32, tag="zraw")
                    cp_i = nc.vector.tensor_copy(zraw[:], zp[:, :, :])

                    rb = rwork.tile([DK, 2, NQ], f32, tag="rb")
                    if qt == QT - 1 and mo == MO - 1:
                        # tail unit: the exp stream is over, so the scalar
                        # engine is free — 1/d = exp(-ln(d)) on the
                        # matmul-replicated denominator rows, with no DMA
                        # round trips on the critical chain (costs two ACT
                        # table loads, but still beats the DRAM bounce).
                        ln_i = nc.scalar.activation(
                            rb[:], zraw[DK:P, :, :],
                            mybir.ActivationFunctionType.Ln)
                        ex_i = nc.scalar.activation(
                            rb[:], rb[:],
                            mybir.ActivationFunctionType.Exp, scale=-1.0)
                        tail_pins = [cp_i, ln_i, ex_i]
                    else:
                        # softmax denominators: spread across lanes via DRAM
                        # for a cheap reciprocal, broadcast back, normalize.
                        NJ = 2 * NQ // P
                        d_sp = rwork.tile([P, NJ], f32, tag="dsp")
                        nc.gpsimd.dma_start(
                            d_sp[:], zraw[DK:DK + 1, :, :])
                        r_sp = rwork.tile([P, NJ], f32, tag="rsp")
                        nc.vector.reciprocal(r_sp[:], d_sp[:])
                        r_dr = dscr.tile([2, NQ], f32, tag="rd")
                        nc.sync.dma_start(
                            r_dr.rearrange("h (a b) -> (h a) b", b=NJ),
                            r_sp[:])
                        nc.sync.dma_start(
                            rb[:], r_dr[None].to_broadcast((DK, 2, NQ)))
                    # engines support differing in/out partition bases, so
                    # h=1 writes its zT half directly (no SBUF-SBUF bounce)
                    nc.vector.tensor_mul(
                        zT_sb[DK:P, mo, ts(qt, NQ)], zraw[0:DK, 1, :],
                        rb[:, 1, :])
                    nc.vector.tensor_mul(
                        zT_sb[0:DK, mo, ts(qt, NQ)], zraw[0:DK, 0, :],
                        rb[:, 0, :])
                prev_qt = qt

            # pinned chunks execute inside the final normalization chain's
            # window so the PE never idles long enough to re-throttle; the
            # last slab's own chunks (plus anything a degenerate mask kept
            # from riding along) close the kernel.
            for k, (sl, j) in enumerate(pinned):
                pin = tail_pins[k % len(tail_pins)] if tail_pins else None
                outproj_chunk(sl, j, cp="vector", pin_after=pin)
            for qt_d in range(QT - 1):
                for mo8 in range(D // P):
                    if (qt_d, mo8) not in done_chunks:
                        outproj_chunk(qt_d, mo8)
            for mo8 in range(D // P):
                outproj_chunk(prev_qt, mo8,
                              cp=("vector" if mo8 % 2 else "scalar"))

    return nc


def _get_program(mask):
    cls, qoff, mixed_idx, mixed_tiles, use_affine = _classify_mask(mask)
    key = (use_affine,
           tuple(tuple(r) for r in cls),
           tuple(tuple(r) for r in qoff))
    if key not in _cache:
        nc = _build_program(cls, qoff, mixed_idx, len(mixed_tiles), use_affine)
        nc.compile()
        _cache[key] = nc
    return _cache[key], mixed_tiles


def _prep_in_maps(x, mask, Wq, bq, Wk, bk, Wv, bv, Wo, bo, mixed_tiles):
    xT = [np.ascontiguousarray(x[b].T).astype(BF16) for b in range(B)]
    in_maps = []
    for core in range(NCORES):
        b, g = divmod(core, GROUPS)
        c0, c1 = g * DG, (g + 1) * DG
        im = {
            "xT": xT[b],
            "wq": np.ascontiguousarray(Wq[:, c0:c1] * SCALE).astype(BF16),
            "wk": np.ascontiguousarray(Wk[:, c0:c1]).astype(BF16),
            "wv": np.ascontiguousarray(Wv[:, c0:c1]).astype(BF16),
            "wo": np.ascontiguousarray(Wo[c0:c1, :]).astype(BF16),
            "bqk": np.ascontiguousarray(
                np.stack([bq[c0:c1] * SCALE, bk[c0:c1]])).astype(np.float32),
        }
        if len(mixed_tiles):
            im["mmask"] = mixed_tiles
        in_maps.append(im)
    return in_maps


def _unshard(results, Wo, bv, bo):
    bo_eff = (bo.astype(np.float32)
              + bv.astype(np.float32) @ Wo.astype(np.float32))
    out = np.empty((B, S, D), np.float32)
    for b in range(B):
        acc = results[b * GROUPS]["outT"].astype(np.float32)
        for g in range(1, GROUPS):
            acc += results[b * GROUPS + g]["outT"].astype(np.float32)
        out[b] = acc.T + bo_eff
    return out


def kernel(trace=False, **inputs):
    from concourse import bass_utils

    args = {k: np.asarray(v) for k, v in inputs.items()}
    x, mask = args["x"], args["mask"]
    Wq, bq = args["Wq"], args["bq"]
    Wk, bk = args["Wk"], args["bk"]
    Wv, bv = args["Wv"], args["bv"]
    Wo, bo = args["Wo"], args["bo"]

    nc, mixed_tiles = _get_program(mask)
    in_maps = _prep_in_maps(x, mask, Wq, bq, Wk, bk, Wv, bv, Wo, bo,
                            mixed_tiles)
    res = bass_utils.run_bass_kernel_spmd(
        nc, in_maps, core_ids=list(range(NCORES)), trace=trace)
    out = _unshard(res.results, Wo, bv, bo)
    kernel.last_results = res
    return out


# revision 54
# speedup vs baseline: 1.0200x; 1.0200x over previous
"""Multi-head attention (B=2, S=2048, D=1024, H=16) on 8 Trainium2 cores.

Sharding: core = b*4 + g  ->  batch b (data parallel), head-group g of 4
heads (tensor parallel).  Each core computes a partial out^T = Wo_g^T @ Z_g
for its batch; the host sums the 4 partials per batch (the "all-reduce"),
transposes back and adds the (folded) output bias.

All activations flow feature-major on device (x^T, Q^T, K^T, scores^T) so
no on-device transposes are needed.  Matmuls run in bf16 with fp32 PSUM
accumulation.  Softmax skips the row-max pass (scores are bounded), gets
its denominator from a ones-column appended to V, and defers normalization
to after the attention*V matmul.  Output partials are written in bf16
(summed in fp32 on the host).
"""

import numpy as np
import ml_dtypes

B, S, D, H = 2, 2048, 1024, 16
DK = D // H                  # 64
SCALE = 1.0 / np.sqrt(D)
NCORES = 8
GROUPS = 4                   # head-groups (tensor parallel)
HG = H // GROUPS             # 4 heads per group
DG = D // GROUPS             # 256 head dims per group
P = 128
KO = D // P                  # 8 contraction chunks for the projections
MO = DG // P                 # 2 row-chunks of Q^T/K^T (= head pairs)
NQ = 512                     # q tile width
QT = S // NQ                 # 4
ST = S // P                  # 16 key blocks / s chunks
BF16 = ml_dtypes.bfloat16

_cache = {}


def _classify_mask(mask):
    """Block structure of mask^T ([k, q] layout, P x NQ blocks).

    Returns (cls, qoff, mixed_idx, mixed_tiles, use_affine):
      cls[kt][qt]  : 0 all-masked, 1 all-kept, 2 mixed
      qoff[kt][qt] : leading all-masked columns (trim), 0 unless tril
      mixed_idx    : {(kt, qt): index into mixed_tiles}
      mixed_tiles  : np [n, P, NQ] bf16 0/1 tiles (empty when use_affine)
    """
    tril = np.tril(np.ones((S, S), dtype=mask.dtype))
    use_affine = bool(np.array_equal(mask, tril))
    cls = [[1] * QT for _ in range(ST)]
    qoff = [[0] * QT for _ in range(ST)]
    mixed_idx = {}
    tiles = []
    if use_affine:
        for kt in range(ST):
            k0 = kt * P
            for qt in range(QT):
                q0 = qt * NQ
                if k0 - q0 >= NQ:
                    cls[kt][qt] = 0
                elif k0 + P - 1 > q0:
                    cls[kt][qt] = 2
                    qoff[kt][qt] = min(max(k0 - q0, 0), NQ - P)
                # else: fully kept
    else:
        keepT = (mask != 0).T        # [k, q]
        for kt in range(ST):
            for qt in range(QT):
                blk = keepT[kt * P:(kt + 1) * P, qt * NQ:(qt + 1) * NQ]
                if not blk.any():
                    cls[kt][qt] = 0
                elif blk.all():
                    cls[kt][qt] = 1
                else:
                    cls[kt][qt] = 2
                    mixed_idx[(kt, qt)] = len(tiles)
                    tiles.append(blk.astype(BF16))
    mixed_tiles = (np.stack(tiles) if tiles else
                   np.zeros((0, P, NQ), dtype=BF16))
    return cls, qoff, mixed_idx, mixed_tiles, use_affine


def _build_program(cls, qoff, mixed_idx, n_mixed, use_affine):
    from contextlib import ExitStack
    import concourse.bass as bass
    import concourse.tile as tile
    import concourse.mybir as mybir
    from concourse import bacc
    from concourse.bass import ds, ts

    f32 = mybir.dt.float32
    bf16 = mybir.dt.bfloat16
    Exp = mybir.ActivationFunctionType.Exp

    nc = bacc.Bacc(None, target_bir_lowering=False, name="mha_tp")

    xT = nc.dram_tensor("xT", [D, S], bf16, kind="ExternalInput")
    wq = nc.dram_tensor("wq", [D, DG], bf16, kind="ExternalInput")
    wk = nc.dram_tensor("wk", [D, DG], bf16, kind="ExternalInput")
    wv = nc.dram_tensor("wv", [D, DG], bf16, kind="ExternalInput")
    wo = nc.dram_tensor("wo", [DG, D], bf16, kind="ExternalInput")
    bqk = nc.dram_tensor("bqk", [2, DG], f32, kind="ExternalInput")
    mm = (nc.dram_tensor("mmask", [n_mixed, P, NQ], bf16, kind="ExternalInput")
          if n_mixed else None)
    outT = nc.dram_tensor("outT", [D, S], bf16, kind="ExternalOutput")

    xTv = xT.ap().rearrange("(ko p) s -> p ko s", p=P)
    wqv = wq.ap().rearrange("(ko p) m -> p ko m", p=P)
    wkv = wk.ap().rearrange("(ko p) m -> p ko m", p=P)
    wvv = wv.ap().rearrange("(ko p) m -> p ko m", p=P)
    wov = wo.ap().rearrange("(zo p) n -> p zo n", p=P)
    bqkv = bqk.ap().rearrange("t (mo p) -> p t mo", p=P)
    outv = outT.ap().rearrange("(mo p) s -> p mo s", p=P)

    with tile.TileContext(nc) as tc, ExitStack() as ctx:
        const = ctx.enter_context(tc.tile_pool(name="const", bufs=1))

        # DMA order matters: the first QKV matmul needs only wq[ko=0] and
        # x[ko=0] of slab 0, so stream those in fine-grained chunks.
        # Descriptor generation (~0.55us per dma_start) serializes per
        # engine queue, so weights go through the (idle at startup) gpsimd
        # queue while x streams through the sync queue in parallel.
        wq_sb = const.tile([P, KO, DG], bf16)
        x_sb = const.tile([P, KO, S], bf16)
        wk_sb = const.tile([P, KO, DG], bf16)
        for ko in range(0, KO, 2):
            nc.sync.dma_start(wq_sb[:, ko:ko + 2, :], wqv[:, ko:ko + 2, :])
            nc.sync.dma_start(x_sb[:, ko:ko + 2, ts(0, NQ)],
                              xTv[:, ko:ko + 2, ts(0, NQ)])
            nc.sync.dma_start(wk_sb[:, ko:ko + 2, :], wkv[:, ko:ko + 2, :])
        bias_sb = const.tile([P, 2, 2], f32)
        nc.gpsimd.dma_start(bias_sb[:], bqkv)
        wv_sb = const.tile([P, KO, DG], bf16)
        wo_sb = const.tile([P, MO, D], bf16)
        for qt in range(1, QT):
            for half in (0, 1):
                ko0 = half * (KO // 2)
                nc.scalar.dma_start(
                    x_sb[:, ko0:ko0 + KO // 2, ts(qt, NQ)],
                    xTv[:, ko0:ko0 + KO // 2, ts(qt, NQ)])
            if qt == 1:
                nc.scalar.dma_start(wv_sb[:, :KO // 2], wvv[:, :KO // 2])
                nc.scalar.dma_start(wv_sb[:, KO // 2:], wvv[:, KO // 2:])
                nc.scalar.dma_start(wo_sb[:], wov)
        mask_sb = None
        if n_mixed:
            mask_sb = const.tile([P, n_mixed, NQ], bf16)
            for i in range(n_mixed):
                nc.sync.dma_start(mask_sb[:, i, :], mm.ap()[i])

        qT_sb = const.tile([P, MO, S], bf16)
        kT_sb = const.tile([P, MO, S], bf16)
        # 64 ones-columns: the AV matmul replicates the softmax denominator
        # across output partitions 64..127 at no extra PE cost (cost is
        # N-bound), which lets the tail compute 1/d = exp(-ln(d)) on the
        # (then idle) scalar engine with no lane-spread DMA round trips.
        v_sb = const.tile([P, ST, HG, 2 * DK], bf16)
        zT_sb = const.tile([P, MO, S], bf16)
        nc.gpsimd.memset(v_sb[:, :, :, DK:], 1.0)

        with (
            tc.tile_pool(name="pqkv", bufs=2, space="PSUM") as pqkv,
            tc.tile_pool(name="ps_at", bufs=2, space="PSUM") as ps_at,
            tc.tile_pool(name="pz", bufs=1, space="PSUM") as pz,
            tc.tile_pool(name="work", bufs=8) as work,
            tc.tile_pool(name="rwork", bufs=4) as rwork,
            tc.tile_pool(name="dscr", bufs=3, space="DRAM") as dscr,
        ):
            def av(zp, mo, prev, last):
                kt, pT, off, first = prev
                ret = None
                for h in (0, 1):
                    ret = nc.tensor.matmul(
                        zp[:, h, off:],
                        v_sb[:, kt, 2 * mo + h, :],
                        pT[:, h, off:],
                        start=first, stop=last)
                return ret

            def outproj_chunk(qt, mo8, cp=None, pin_after=None):
                o_ps = pqkv.tile([P, NQ], f32, tag="ps", name=f"o{mo8}")
                for zo in range(MO):
                    mm = nc.tensor.matmul(
                        o_ps, wo_sb[:, zo, ts(mo8, P)],
                        zT_sb[:, zo, ts(qt, NQ)],
                        start=(zo == 0), stop=(zo == MO - 1))
                    if zo == 0 and pin_after is not None:
                        tile.add_dep_helper(
                            mm.ins, pin_after.ins,
                            reason="pin deferred outproj into tail hole")
                o_sb = work.tile([P, NQ], bf16, tag="osb")
                if cp is None:
                    cp = "vector"
                if cp == "vector":
                    nc.vector.tensor_copy(o_sb[:], o_ps)
                else:
                    nc.scalar.copy(o_sb[:], o_ps)
                nc.sync.dma_start(outv[:, mo8, ts(qt, NQ)], o_sb[:])

            vps_live = {}
            qkps_live = {}

            def qk_piece(sl, t, mo, kp):
                # one quarter of a Q- or K-projection chunk of slab `sl`,
                # rideable inside an earlier attention stream
                w_sb, dst = ((wq_sb, qT_sb), (wk_sb, kT_sb))[t]
                key = (sl, t, mo)
                if kp == 0:
                    qkps_live[key] = pqkv.tile([P, NQ], f32, tag="ps",
                                               name=f"qk{sl}{t}{mo}")
                ps = qkps_live[key]
                for ko in (2 * kp, 2 * kp + 1):
                    nc.tensor.matmul(
                        ps, w_sb[:, ko, ts(mo, P)],
                        x_sb[:, ko, ts(sl, NQ)],
                        start=(ko == 0), stop=(ko == KO - 1))
                if kp == 3:
                    nc.vector.tensor_scalar_add(
                        dst[:, mo, ts(sl, NQ)], ps,
                        bias_sb[:, t, mo:mo + 1])
                    del qkps_live[key]

            def v_piece(so, kp):
                # one quarter of a V-projection s-block: rideable filler
                # that keeps each insertion below the attention stream's
                # per-kt PE slack.  The PSUM accumulator lives across the
                # four pieces.
                if kp == 0:
                    vps_live[so] = pqkv.tile([P, NQ], f32, tag="ps",
                                             name=f"v{so}")
                ps = vps_live[so]
                for ko in (2 * kp, 2 * kp + 1):
                    nc.tensor.matmul(
                        ps[:, :DG], x_sb[:, ko, ts(so, P)],
                        wv_sb[:, ko, :],
                        start=(ko == 0), stop=(ko == KO - 1))
                if kp == 3:
                    nc.vector.tensor_copy(
                        v_sb[:, so, :, 0:DK],
                        ps[:, :DG].rearrange("p (h d) -> p h d", h=HG))
                    del vps_live[so]

            v_ridden = set()
            qk_ridden = set()

            def qkv_slab(qt):
                if qt not in qk_ridden:
                    for t in range(2):
                        for mo in range(MO):
                            for kp in range(4):
                                qk_piece(qt, t, mo, kp)
                if qt in v_ridden:
                    return
                for so in range(HG * qt, HG * (qt + 1)):
                    for kp in range(4):
                        v_piece(so, kp)

            if not use_affine:
                # a general mask may attend beyond block qt, so all K/V
                # slabs must exist before any attention starts
                for qt in range(QT):
                    qkv_slab(qt)

            proc = list(range(QT))
            emitted = 0
            prev_qt = None
            tail_pins = []        # instructions of the last unit's norm chain
            # ride-along plan: which deferred out-projection chunks fill the
            # PE slack of each (qt, mo) attention stream (ACT-exp-bound, so
            # the PE has ~400ns/kt spare).  The last slab's two streams have
            # no QKV emission to overlap, so they get most of the inventory;
            # two chunks stay pinned for the final normalization chain.
            ride = {}
            ride_late = set()     # keys whose rides wait for mid-stream deps
            pinned = []
            if QT == 4:
                for q in (1, 2, 3):
                    ride[(q, 1)] = [("o", q - 1, j) for j in range(4)]
                ride[(3, 0)] = [("o", 2, j) for j in range(4)]
                ride_late.add((3, 0))
                ride[(3, 1)] += [("o", 0, j) for j in range(4, 7)] \
                    + [("o", 1, j) for j in range(4, 7)] \
                    + [("o", 2, j) for j in range(4, 6)]
                pinned = [(0, 7), (1, 7), (2, 6), (2, 7)]
                if use_affine:
                    # the V-projections of slabs 2/3 and all of slab 3's
                    # Q/K ride in the previous slab's attention streams,
                    # shrinking the exp-starved PE-only QKV phases
                    ride[(1, 0)] = [("v", so, kp)
                                    for so in range(2 * HG, 3 * HG)
                                    for kp in range(4)]
                    ride[(2, 0)] = [("v", so, kp)
                                    for so in range(3 * HG, 4 * HG)
                                    for kp in range(4)]
                    v_ridden.update((2, 3))
                    ride[(2, 1)] += [("q", (3, t, mo), kp)
                                     for t in range(2) for mo in range(MO)
                                     for kp in range(4)]
                    qk_ridden.add(3)
            done_chunks = set(pinned)
            for qt in proc:
                q0 = qt * NQ
                if use_affine:
                    # attention(qt) only needs k blocks <= qt, so emit QKV
                    # slabs lazily just ahead of it
                    while emitted <= qt:
                        qkv_slab(emitted)
                        emitted += 1

                # -- attention over k blocks of this slab -----------------
                for mo in range(MO):
                    # descending: diagonal (affine-masked) blocks first, so
                    # their extra exp->mask->AV latency overlaps the stream
                    # warm-up instead of draining exposed at the end
                    kts = [kt for kt in range(ST) if cls[kt][qt] != 0][::-1]
                    if not kts:
                        nc.vector.memset(zT_sb[:, mo, ts(qt, NQ)], 0.0)
                        continue
                    zp = pz.tile([P, 2, NQ], f32, tag="z")
                    prev = None
                    for i, kt in enumerate(kts):
                        k0 = kt * P
                        off = qoff[kt][qt]
                        w = NQ - off
                        pT = work.tile([P, 2, NQ], bf16, tag="pT")
                        s_ps = ps_at.tile([P, 2, NQ], f32, tag="s")
                        for h in (0, 1):
                            hp = slice(h * DK, (h + 1) * DK)
                            nc.tensor.matmul(
                                s_ps[:, h, off:],
                                kT_sb[hp, mo, ts(kt, P)],
                                qT_sb[hp, mo, ds(q0 + off, w)],
                                start=True, stop=True,
                                tile_position=(h * DK, 0))
                        nc.scalar.activation(
                            pT[:, :, off:], s_ps[:, :, off:], Exp)
                        if cls[kt][qt] == 2:
                            if use_affine:
                                nc.gpsimd.affine_select(
                                    out=pT[:, :, off:],
                                    in_=pT[:, :, off:],
                                    compare_op=mybir.AluOpType.is_ge,
                                    fill=0.0,
                                    base=q0 + off - k0,
                                    channel_multiplier=-1,
                                    pattern=[[0, 2], [1, w]])
                            else:
                                nc.vector.tensor_mul(
                                    pT[:, :, off:], pT[:, :, off:],
                                    mask_sb[:, mixed_idx[(kt, qt)], None,
                                            off:].to_broadcast((P, 2, w)))
                        if prev is not None:
                            av(zp, mo, prev, last=False)
                        prev = (kt, pT, off, i == 0)
                        rl = ride.get((qt, mo), [])
                        if rl:
                            # late-dep rides (outproj needing a zT that
                            # lands mid-stream) spread over the second half
                            i0 = (len(kts) // 2
                                  if (qt, mo) in ride_late else 0)
                            if i >= i0:
                                span = max(len(kts) - i0, 1)
                                lo = (i - i0) * len(rl) // span
                                hi = (i - i0 + 1) * len(rl) // span
                                for kind, a, b in rl[lo:hi]:
                                    if kind == "o":
                                        outproj_chunk(a, b)
                                        done_chunks.add((a, b))
                                    elif kind == "v":
                                        v_piece(a, b)
                                    else:
                                        qk_piece(a[0], a[1], a[2], b)
                    last_av = av(zp, mo, prev, last=True)

                    # Copy raw z out of PSUM immediately (frees the bank
                    # for the next tile); normalization below is then fully
                    # asynchronous with the attention stream.
                    zraw = rwork.tile([P, 2, NQ], f32, tag="zraw")
                    cp_i = nc.vector.tensor_copy(zraw[:], zp[:, :, :])

                    rb = rwork.tile([DK, 2, NQ], f32, tag="rb")
                    if qt == QT - 1 and mo == MO - 1:
                        # tail unit: the exp stream is over, so the scalar
                        # engine is free — 1/d = exp(-ln(d)) on the
                        # matmul-replicated denominator rows, with no DMA
                        # round trips on the critical chain (costs two ACT
                        # table loads, but still beats the DRAM bounce).
                        ln_i = nc.scalar.activation(
                            rb[:], zraw[DK:P, :, :],
                            mybir.ActivationFunctionType.Ln)
                        ex_i = nc.scalar.activation(
                            rb[:], rb[:],
                            mybir.ActivationFunctionType.Exp, scale=-1.0)
                        tail_pins = [cp_i, ln_i, ex_i]
                    else:
                        # softmax denominators: spread across lanes via DRAM
                        # for a cheap reciprocal, broadcast back, normalize.
                        NJ = 2 * NQ // P
                        d_sp = rwork.tile([P, NJ], f32, tag="dsp")
                        nc.gpsimd.dma_start(
                            d_sp[:], zraw[DK:DK + 1, :, :])
                        r_sp = rwork.tile([P, NJ], f32, tag="rsp")
                        nc.vector.reciprocal(r_sp[:], d_sp[:])
                        r_dr = dscr.tile([2, NQ], f32, tag="rd")
                        nc.sync.dma_start(
                            r_dr.rearrange("h (a b) -> (h a) b", b=NJ),
                            r_sp[:])
                        nc.sync.dma_start(
                            rb[:], r_dr[None].to_broadcast((DK, 2, NQ)))
                    # engines support differing in/out partition bases, so
                    # h=1 writes its zT half directly (no SBUF-SBUF bounce)
                    nc.vector.tensor_mul(
                        zT_sb[DK:P, mo, ts(qt, NQ)], zraw[0:DK, 1, :],
                        rb[:, 1, :])
                    nc.vector.tensor_mul(
                        zT_sb[0:DK, mo, ts(qt, NQ)], zraw[0:DK, 0, :],
                        rb[:, 0, :])
                prev_qt = qt

            # pinned chunks execute inside the final normalization chain's
            # window so the PE never idles long enough to re-throttle; the
            # last slab's own chunks (plus anything a degenerate mask kept
            # from riding along) close the kernel.
            for k, (sl, j) in enumerate(pinned):
                pin = tail_pins[k % len(tail_pins)] if tail_pins else None
                outproj_chunk(sl, j, cp="vector", pin_after=pin)
            for qt_d in range(QT - 1):
                for mo8 in range(D // P):
                    if (qt_d, mo8) not in done_chunks:
                        outproj_chunk(qt_d, mo8)
            for mo8 in range(D // P):
                outproj_chunk(prev_qt, mo8,
                              cp=("vector" if mo8 % 2 else "scalar"))

    return nc


def _get_program(mask):
    cls, qoff, mixed_idx, mixed_tiles, use_affine = _classify_mask(mask)
    key = (use_affine,
           tuple(tuple(r) for r in cls),
           tuple(tuple(r) for r in qoff))
    if key not in _cache:
        nc = _build_program(cls, qoff, mixed_idx, len(mixed_tiles), use_affine)
        nc.compile()
        _cache[key] = nc
    return _cache[key], mixed_tiles


def _prep_in_maps(x, mask, Wq, bq, Wk, bk, Wv, bv, Wo, bo, mixed_tiles):
    xT = [np.ascontiguousarray(x[b].T).astype(BF16) for b in range(B)]
    in_maps = []
    for core in range(NCORES):
        b, g = divmod(core, GROUPS)
        c0, c1 = g * DG, (g + 1) * DG
        im = {
            "xT": xT[b],
            "wq": np.ascontiguousarray(Wq[:, c0:c1] * SCALE).astype(BF16),
            "wk": np.ascontiguousarray(Wk[:, c0:c1]).astype(BF16),
            "wv": np.ascontiguousarray(Wv[:, c0:c1]).astype(BF16),
            "wo": np.ascontiguousarray(Wo[c0:c1, :]).astype(BF16),
            "bqk": np.ascontiguousarray(
                np.stack([bq[c0:c1] * SCALE, bk[c0:c1]])).astype(np.float32),
        }
        if len(mixed_tiles):
            im["mmask"] = mixed_tiles
        in_maps.append(im)
    return in_maps


def _unshard(results, Wo, bv, bo):
    bo_eff = (bo.astype(np.float32)
              + bv.astype(np.float32) @ Wo.astype(np.float32))
    out = np.empty((B, S, D), np.float32)
    for b in range(B):
        acc = results[b * GROUPS]["outT"].astype(np.float32)
        for g in range(1, GROUPS):
            acc += results[b * GROUPS + g]["outT"].astype(np.float32)
        out[b] = acc.T + bo_eff
    return out


def kernel(trace=False, **inputs):
    from concourse import bass_utils

    args = {k: np.asarray(v) for k, v in inputs.items()}
    x, mask = args["x"], args["mask"]
    Wq, bq = args["Wq"], args["bq"]
    Wk, bk = args["Wk"], args["bk"]
    Wv, bv = args["Wv"], args["bv"]
    Wo, bo = args["Wo"], args["bo"]

    nc, mixed_tiles = _get_program(mask)
    in_maps = _prep_in_maps(x, mask, Wq, bq, Wk, bk, Wv, bv, Wo, bo,
                            mixed_tiles)
    res = bass_utils.run_bass_kernel_spmd(
        nc, in_maps, core_ids=list(range(NCORES)), trace=trace)
    out = _unshard(res.results, Wo, bv, bo)
    kernel.last_results = res
    return out
